# revision 12
# baseline (speedup 1.0000x reference)
"""Expert-choice MoE layer (NucleusMoELayer) on 8 Trainium2 NeuronCores.

Strategy (expert-parallel):
 - one expert per core; router + gate-normalization replicated from an
   AllGathered logit table; shared expert sharded over tokens (1024/core)
 - router logits from host-transposed hsuT shard (no PE transposes) +
   host-computed timestep bias; router MMs spread over early m1 half-units
 - expert-choice top-1024-per-(batch,expert) via 26-step threshold bisection
   (5-op iterations, fused count via accum_out), woven LATE through the
   shared FFN so the PE never head-of-line blocks on the AllGather
 - shared m1 streams packed (a_i|g_i) weight tiles (8KB resident) so the
   expert W1 can be fully resident (64KB) in parallel; expert W2 aliases
   the shared-W2 tiles after their last use
 - DMA queue split: phase-1 streams (w1s/xsT/hT/wrs2) on the Sync HWDGE
   FIFO; resident weights (wr1/wr2), consts and AG stores/loads on the
   scalar/gpsimd queues so they never sit behind ring-gated streams
 - small fp32 matmuls (bisect count, gates, compact, gate-replicate) run
   single-pass via float32r (fp22 keeps the <=2^13 integer counts exact)
 - compaction (selected tokens -> dense slots) via per-partition cumsum +
   GpSimd local_scatter of (token-id, gate-hi, gate-lo) uint16 payloads
 - dispatch: indirect-DMA row gather of selected tokens, swiglu FFN in bf16
 - outputs feature-major bf16; host transposes, upcasts, scatter-adds

kernel(**inputs) takes FULL unsharded inputs, returns the FULL output.
"""

import sys

if "/opt/trn_rl_repo" not in sys.path:
    sys.path.insert(0, "/opt/trn_rl_repo")

import numpy as np

import concourse.bacc as bacc
import concourse.bass as bass
import concourse.mybir as mybir
import concourse.tile as tile
from concourse.bass_utils import run_bass_kernel_spmd

dt = mybir.dt
AF = mybir.ActivationFunctionType
ALU = mybir.AluOpType

NCORES = 8
BS, SLEN, DIM = 2, 4096, 1024
INNER = 2048
I2 = 2 * INNER  # 4096
E = 8
CAP = 1024  # tokens per (batch, expert)
T = BS * SLEN  # 8192 global tokens
TSH = T // NCORES  # 1024-token shard per core
SLOTS = BS * CAP  # 2048 routed slots per expert
KD = DIM // 128  # 8 k-chunks over dim
KI = INNER // 128  # 16 k-chunks over inner
BISECT_ITERS = 26

# packed-constant column layout (constf [128, 306] f32)
C_ESEL, C_DSEL, C_LT16, C_B2, C_RS, C_ONES = 0, 16, 32, 48, 50, 178


def build_nc():
    nc = bacc.Bacc(None, target_bir_lowering=False, num_devices=NCORES)

    tens = {}

    def din(name, shape, dtype=dt.float32):
        tens[name] = nc.dram_tensor(name, shape, dtype, kind="ExternalInput")

    def dout(name, shape, dtype=dt.float32):
        tens[name] = nc.dram_tensor(name, shape, dtype, kind="ExternalOutput")

    din("hs_b", [T, DIM], dt.bfloat16)
    din("hs_shT_b", [DIM, TSH], dt.bfloat16)
    din("hsuT_sh", [DIM, TSH])
    din("wgtp", [128, KD * E])
    din("bias_mine", [E, 1])
    din("w1_b", [DIM, I2], dt.bfloat16)  # expert W1, resident
    din("w1p_b", [DIM, I2], dt.bfloat16)  # shared W1, (a_i|g_i)-packed cols
    din("w2_b", [INNER, DIM], dt.bfloat16)  # expert W2
    din("ws2_b", [INNER, DIM], dt.bfloat16)  # shared W2, resident
    din("constf", [128, 306])
    din("consth", [128, 304], dt.float16)
    din("iota_tid", [16, 512], dt.uint16)
    dout("out_routed", [DIM, SLOTS], dt.bfloat16)
    dout("out_idx", [SLOTS, 1], dt.int32)
    dout("out_shared", [DIM, TSH], dt.bfloat16)
    dout("dbg_thr", [128, 1])
    dout("dbg_gate", [BS, CAP])

    with tile.TileContext(nc, num_cores=NCORES) as tc:
        _emit(nc, tc, tens)
    nc.finalize()
    return nc


def _emit(nc, tc, t):
    from contextlib import ExitStack

    ctx = ExitStack()
    with ctx:
        const = ctx.enter_context(tc.tile_pool(name="const", bufs=1))
        sb = ctx.enter_context(tc.tile_pool(name="sb", bufs=2))
        sb1 = ctx.enter_context(tc.tile_pool(name="sb1", bufs=1))
        ws = ctx.enter_context(tc.tile_pool(name="ws", bufs=1))
        hTp = ctx.enter_context(tc.tile_pool(name="hTp", bufs=1))
        cw = ctx.enter_context(tc.tile_pool(name="cw", bufs=1))
        bis = ctx.enter_context(tc.tile_pool(name="bis", bufs=1))
        dr = ctx.enter_context(tc.tile_pool(name="dr", bufs=1, space="DRAM"))
        pmm = ctx.enter_context(tc.tile_pool(name="pmm", bufs=6, space="PSUM"))
        psm = ctx.enter_context(tc.tile_pool(name="psm", bufs=2, space="PSUM"))

        # ---- consts (scalar HWDGE queue; off the phase-1 Sync FIFO) ----
        cf = const.tile([128, 306], dt.float32, tag="constf")
        nc.scalar.dma_start(cf[:], t["constf"][:])
        iota_tid = const.tile([16, 512], dt.uint16, tag="iota")
        nc.scalar.dma_start(iota_tid[:], t["iota_tid"][:])
        wgt_sb = const.tile([128, KD * E], dt.float32, tag="wgt")
        nc.scalar.dma_start(wgt_sb[:], t["wgtp"][:])
        bias_sb = const.tile([E, 1], dt.float32, tag="bias")
        nc.scalar.dma_start(bias_sb[:], t["bias_mine"][:])
        ch = const.tile([128, 304], dt.float16, tag="consth")
        nc.scalar.dma_start(ch[:], t["consth"][:])

        b2 = cf[:16, C_B2 : C_B2 + 2]
        rsmat = ch[:, 0:128]
        esel = ch[:, 128:144]
        dsel = ch[:, 144:160]
        lt16 = ch[:16, 160:176]
        ones128 = ch[:1, 176:304]

        # shared-expert input: host-transposed shard, one batched DMA per half
        hsT_src = t["hs_shT_b"][:].rearrange("(k p) n -> p k n", p=128)
        xsT = []

        def xsT_dma(n):
            xt = sb1.tile([128, KD * 512], dt.bfloat16, tag=f"xT{n}")
            nc.sync.dma_start(
                xt[:].rearrange("p (k t) -> p k t", t=512),
                hsT_src[:, :, n * 512 : (n + 1) * 512],
            )
            xsT.append(xt)

        # ---------------- units ----------------
        # shared m1, i-outer / chunk-inner, streamed packed weights
        w1s = {}
        w1p_src = t["w1p_b"][:].rearrange("(k p) c -> p k c", p=128)

        def w1s_dma(i):
            wt = ws.tile([128, KD * 256], dt.bfloat16, tag=f"w1s{i % 2}")
            nc.sync.dma_start(
                wt[:].rearrange("p (k c) -> p k c", c=256),
                w1p_src[:, :, i * 256 : (i + 1) * 256],
            )
            w1s[i] = wt

        h_sh = []
        for n in range(2):
            h_tile = sb1.tile([128, KI * 512], dt.bfloat16, tag=f"h_sb{n}")
            h_sh.append(h_tile)

        def sh_m1_half(i, n):
            def f():
                if n == 0 and i + 1 < 16:
                    w1s_dma(i + 1)
                wt = w1s[i]
                ps_a = pmm.tile([128, 512], dt.float32, tag="mm")
                for k in range(KD):
                    nc.tensor.matmul(
                        ps_a[:],
                        lhsT=wt[:, k * 256 : k * 256 + 128],
                        rhs=xsT[n][:, k * 512 : (k + 1) * 512],
                        start=(k == 0),
                        stop=(k == KD - 1),
                    )
                ps_g = pmm.tile([128, 512], dt.float32, tag="mm")
                for k in range(KD):
                    nc.tensor.matmul(
                        ps_g[:],
                        lhsT=wt[:, k * 256 + 128 : (k + 1) * 256],
                        rhs=xsT[n][:, k * 512 : (k + 1) * 512],
                        start=(k == 0),
                        stop=(k == KD - 1),
                    )
                sl = sb1.tile([128, 512], dt.bfloat16, tag=f"silu{i % 2}")
                nc.scalar.activation(sl[:], ps_g[:], AF.Silu)
                nc.vector.tensor_mul(
                    h_sh[n][:, i * 512 : (i + 1) * 512], ps_a[:], sl[:]
                )

            return f

        # resident weights: expert W1 (separate), shared W2 (aliased by W2 later)
        wr1 = [None] * KD
        wrs2 = [None] * KI
        wr2 = [None] * KI

        def mk_wload(dst, idx, src, width, tagp, q):
            def f():
                wt = ws.tile([128, width], dt.bfloat16, tag=f"{tagp}{idx}")
                q.dma_start(wt[:], src[idx * 128 : (idx + 1) * 128, :])
                dst[idx] = wt

            return f

        # ---- router: 2 accumulation units of 4 k-chunks per token half ----
        ag_in = dr.tile([E, TSH], dt.float32)
        hT_t = {}
        lps_t = {}

        def mk_router_dma(n, ks):
            def f():
                for k in ks:
                    ht = hTp.tile([128, 512], dt.float32, tag=f"hT{k % 4}")
                    nc.sync.dma_start(
                        ht[:],
                        t["hsuT_sh"][
                            k * 128 : (k + 1) * 128, n * 512 : (n + 1) * 512
                        ],
                    )
                    hT_t[(n, k)] = ht

            return f

        def mk_router_mm(n, part, ks):
            def f():
                if part == 0:
                    lps = psm.tile([128, 512], dt.float32, tag="small")
                    lps_t[n] = lps
                lps = lps_t[n]
                for j, k in enumerate(ks):
                    nc.tensor.matmul(
                        lps[:E, :],
                        lhsT=wgt_sb[:, k * E : (k + 1) * E],
                        rhs=hT_t[(n, k)][:],
                        start=(part == 0 and j == 0),
                        stop=(part == 1 and j == len(ks) - 1),
                        skip_group_check=True,
                    )

            return f

        def mk_router_fin(n):
            def f():
                lchunk = cw.tile([E, 512], dt.float32, tag="cwb")
                nc.vector.tensor_scalar(
                    lchunk[:], lps_t[n][:E, :], bias_sb[:], None, op0=ALU.add
                )
                nc.gpsimd.dma_start(ag_in[:, n * 512 : (n + 1) * 512], lchunk[:])

            return f

        ag_out = dr.tile([NCORES * E, TSH], dt.float32, addr_space="Shared")
        logit_all = sb1.tile([128, 512], dt.float32, tag="logit_all")
        sig = sb1.tile([128, 512], dt.float16, tag="sig")

        def rt_collective():
            nc.gpsimd.collective_compute(
                "AllGather",
                ALU.bypass,
                replica_groups=[list(range(NCORES))],
                ins=[ag_in[:]],
                outs=[ag_out[:]],
            )
            nc.gpsimd.dma_start(
                logit_all[:],
                ag_out[:].rearrange("(r e) (c t) -> (r e c) t", e=E, c=2),
            )

        def rt_sig():
            nc.scalar.activation(sig[:], logit_all[:], AF.Sigmoid)

        # bisection: lo converges to the top-CAP threshold in logit space
        lo = sb1.tile([128, 1], dt.float32, tag="lo")

        def rt_init():
            nc.vector.memset(lo[:], -16.0)

        def mk_bisect(it):
            step = 32.0 / (2.0 ** (it + 1))

            def f():
                mid = bis.tile([128, 1], dt.float32, tag="mid")
                nc.vector.tensor_scalar(mid[:], lo[:], step, None, op0=ALU.add)
                cmp = bis.tile([128, 512], dt.bfloat16, tag="cmp")
                cnt = bis.tile([128, 1], dt.float16, tag="cnt")
                nc.vector.tensor_scalar(
                    cmp[:], logit_all[:], mid[:], 0.0, op0=ALU.is_ge,
                    op1=ALU.add, accum_out=cnt[:],
                )
                cntg_ps = psm.tile([128, 512], dt.float32, tag="small")
                nc.tensor.matmul(
                    cntg_ps[:, :1], lhsT=rsmat, rhs=cnt[:],
                    start=True, stop=True,
                )
                pred = bis.tile([128, 1], dt.uint8, tag="pred")
                nc.vector.tensor_scalar(
                    pred[:], cntg_ps[:, :1], float(CAP), None, op0=ALU.is_ge
                )
                nc.vector.copy_predicated(lo[:], pred[:], mid[:])

            return f

        gate_t = {}

        def rt_gates():
            nc.scalar.dma_start(t["dbg_thr"][:], lo[:])
            mask = sb1.tile([128, 512], dt.float16, tag="gmask")
            nc.vector.tensor_scalar(
                mask[:], logit_all[:], lo[:], None, op0=ALU.is_ge
            )
            g = sig  # in-place: sig is dead after this
            nc.vector.tensor_mul(g[:], sig[:], mask[:])
            gm_ps = psm.tile([128, 512], dt.float32, tag="small")
            nc.tensor.matmul(
                gm_ps[:16, :], lhsT=esel, rhs=g[:], start=True, stop=True
            )
            dm_ps = psm.tile([128, 512], dt.float32, tag="small")
            nc.tensor.matmul(
                dm_ps[:16, :], lhsT=dsel, rhs=g[:], start=True, stop=True
            )
            dsafe = cw.tile([16, 512], dt.float32, tag="cwa")
            nc.vector.tensor_scalar(
                dsafe[:], dm_ps[:16, :], 1e-12, None, op0=ALU.add
            )
            drec = cw.tile([16, 512], dt.float32, tag="cwb")
            nc.vector.reciprocal_approx_fast(drec[:], dsafe[:])
            ghat_mine = sb1.tile([16, 512], dt.float32, tag="ghat_mine")
            nc.vector.tensor_mul(ghat_mine[:], gm_ps[:16, :], drec[:])
            msk_ps = psm.tile([128, 512], dt.float32, tag="small")
            nc.tensor.matmul(
                msk_ps[:16, :], lhsT=esel, rhs=mask[:],
                start=True, stop=True,
            )
            mask_mine = sb1.tile([16, 512], dt.float32, tag="mask_mine")
            nc.vector.tensor_copy(mask_mine[:], msk_ps[:16, :])
            gate_t["ghat_mine"] = ghat_mine
            gate_t["mask_mine"] = mask_mine

        idx16_buf = dr.tile([SLOTS, 1], dt.int16)
        gate_buf = dr.tile([BS, CAP], dt.float32)

        def rt_compact():
            ghat_mine = gate_t["ghat_mine"]
            mask_mine = gate_t["mask_mine"]
            incl = cw.tile([16, 512], dt.float16, tag="cwf")
            nc.vector.tensor_tensor_scan(
                incl[:], mask_mine[:], mask_mine[:], 0.0,
                op0=ALU.add, op1=ALU.bypass,
            )
            offs_ps = psm.tile([128, 512], dt.float32, tag="small")
            nc.tensor.matmul(
                offs_ps[:16, :1], lhsT=lt16, rhs=incl[:, 511:512],
                start=True, stop=True,
            )
            pos = cw.tile([16, 512], dt.float32, tag="cwh")
            nc.vector.tensor_sub(pos[:], incl[:], mask_mine[:])
            offs = cw.tile([16, 1], dt.float32, tag="cwo")
            nc.vector.tensor_copy(offs[:], offs_ps[:16, :1])
            nc.vector.tensor_scalar(pos[:], pos[:], offs[:], None, op0=ALU.add)
            boff = cw.tile([16, 1], dt.float32, tag="cwo2")
            nc.vector.tensor_scalar(
                boff[:], b2[:, 1:2], float(CAP), None, op0=ALU.mult
            )
            nc.vector.tensor_scalar(pos[:], pos[:], boff[:], None, op0=ALU.subtract)
            okm = cw.tile([16, 512], dt.float32, tag="cwa")
            nc.vector.tensor_scalar(
                okm[:], pos[:], float(CAP - 1), None, op0=ALU.is_le
            )
            nc.vector.tensor_mul(okm[:], okm[:], mask_mine[:])
            p1 = cw.tile([16, 512], dt.float32, tag="cwb")
            nc.vector.tensor_scalar(p1[:], pos[:], 1.0, None, op0=ALU.add)
            nc.vector.tensor_mul(p1[:], p1[:], okm[:])
            nc.vector.tensor_scalar(p1[:], p1[:], 1.0, None, op0=ALU.subtract)
            pos_i16 = sb1.tile([16, 512], dt.int16, tag="pos_i16")
            nc.vector.tensor_copy(pos_i16[:], p1[:])

            gbits = (
                ghat_mine[:].bitcast(dt.uint16).rearrange("p (t two) -> p t two", two=2)
            )
            glo = cw.tile([16, 512], dt.uint16, tag="cwb")
            nc.vector.tensor_copy(glo[:, :, None], gbits[:, :, 0:1])
            ghi = cw.tile([16, 512], dt.uint16, tag="cwa")
            nc.vector.tensor_copy(ghi[:, :, None], gbits[:, :, 1:2])

            # combined per-batch rows; gates written as interleaved u16 halves
            # (lo, hi) so gf bitcasts straight to the packed f32 gate values
            gf = cw.tile([BS, 2 * CAP], dt.uint16, tag="gf")
            gfw = gf[:].rearrange("b (t two) -> b t two", two=2)
            tid_i = cw.tile([BS, CAP], dt.int32, tag="cwh")
            tid_i16 = cw.tile([BS, CAP], dt.int16, tag="cws3")
            for name, data in (("tid", iota_tid), ("ghi", ghi), ("glo", glo)):
                so = cw.tile([16, CAP], dt.uint16, tag="cws2")
                nc.gpsimd.local_scatter(
                    out_ap=so[:],
                    data_ap=data[:],
                    idxs_ap=pos_i16[:],
                    channels=16,
                    num_elems=CAP,
                    num_idxs=512,
                )
                sf = cw.tile([16, CAP], dt.float32, tag="cwf")
                nc.vector.tensor_copy(sf[:], so[:])
                for h in range(2):
                    cps = psm.tile([128, 512], dt.float32, tag="small")
                    nc.tensor.matmul(
                        cps[:BS, :],
                        lhsT=b2,
                        rhs=sf[:, h * 512 : (h + 1) * 512],
                        start=True,
                        stop=True,
                    )
                    hs = slice(h * 512, (h + 1) * 512)
                    if name == "tid":
                        nc.vector.tensor_copy(tid_i[:, hs], cps[:BS, :])
                        nc.vector.tensor_copy(tid_i16[:, hs], cps[:BS, :])
                    elif name == "ghi":
                        nc.vector.tensor_copy(gfw[:, hs, 1:2], cps[:BS, :, None])
                    else:
                        nc.vector.tensor_copy(gfw[:, hs, 0:1], cps[:BS, :, None])

            gatec = gf[:].bitcast(dt.float32)
            nc.sync.dma_start(t["dbg_gate"][:], gatec)
            nc.sync.dma_start(gate_buf[:], gatec)
            nc.sync.dma_start(
                t["out_idx"][:].rearrange("(b t) one -> b (t one)", b=BS), tid_i[:]
            )
            nc.sync.dma_start(
                idx16_buf[:].rearrange("(b t) one -> b (t one)", b=BS), tid_i16[:]
            )

        xe_t = {}
        idx16_w = idx16_buf[:].rearrange("(n c p) one -> p (n c one)", p=16, c=32)

        def mk_gather(n):
            def f():
                idxw = sb.tile([128, 32], dt.int16, tag="idxw")
                for rep in range(8):
                    nc.sync.dma_start(
                        idxw[rep * 16 : (rep + 1) * 16, :],
                        idx16_w[:, n * 32 : (n + 1) * 32],
                    )
                xT = sb1.tile([128, KD * 512], dt.bfloat16, tag=f"xT{n % 2}")
                nc.gpsimd.dma_gather(
                    out_ap=xT[:].rearrange("p (k t) -> p k t", t=512),
                    in_ap=t["hs_b"][:],
                    idxs_ap=idxw[:],
                    num_idxs=512,
                    num_idxs_reg=512,
                    elem_size=DIM,
                    transpose=True,
                )
                xe_t[n] = xT
                grow = sb.tile([1, 512], dt.float32, tag="grow")
                nc.sync.dma_start(
                    grow[:],
                    gate_buf[:].rearrange("b (m t) -> (b m) t", t=512)[n : n + 1, :],
                )
                gate_t[f"grow{n}"] = grow

            return f

        def mk_grep(n):
            def f():
                grow = gate_t[f"grow{n}"]
                g16 = sb.tile([1, 512], dt.float16, tag="g16")
                nc.vector.tensor_copy(g16[:], grow[:])
                grep_ps = psm.tile([128, 512], dt.float32, tag="small")
                nc.tensor.matmul(
                    grep_ps[:], lhsT=ones128, rhs=g16[:],
                    start=True, stop=True,
                )
                gsb = sb1.tile([128, 512], dt.float32, tag=f"gate{n % 2}")
                nc.vector.tensor_copy(gsb[:], grep_ps[:])
                gate_t[f"g{n}"] = gsb

            return f

        # expert FFN units (resident wr1 / wr2)
        def mk_m1(n, i, h_sb):
            def f():
                xT = xe_t[n]
                ps_a = pmm.tile([128, 512], dt.float32, tag="mm")
                for k in range(KD):
                    nc.tensor.matmul(
                        ps_a[:],
                        lhsT=wr1[k][:, i * 128 : (i + 1) * 128],
                        rhs=xT[:, k * 512 : (k + 1) * 512],
                        start=(k == 0),
                        stop=(k == KD - 1),
                    )
                ps_g = pmm.tile([128, 512], dt.float32, tag="mm")
                for k in range(KD):
                    nc.tensor.matmul(
                        ps_g[:],
                        lhsT=wr1[k][:, (16 + i) * 128 : (17 + i) * 128],
                        rhs=xT[:, k * 512 : (k + 1) * 512],
                        start=(k == 0),
                        stop=(k == KD - 1),
                    )
                sl = sb1.tile([128, 512], dt.bfloat16, tag=f"silu{i % 2}")
                nc.scalar.activation(sl[:], ps_g[:], AF.Silu)
                nc.vector.tensor_mul(
                    h_sb[:, i * 512 : (i + 1) * 512], ps_a[:], sl[:]
                )

            return f

        def mk_m2(wr2_l, h_sb, out_dram, out_col, gate_key, mo):
            def f():
                ps2 = pmm.tile([128, 512], dt.float32, tag="mm")
                for k2 in range(KI):
                    nc.tensor.matmul(
                        ps2[:],
                        lhsT=wr2_l[k2][:, mo * 128 : (mo + 1) * 128],
                        rhs=h_sb[:, k2 * 512 : (k2 + 1) * 512],
                        start=(k2 == 0),
                        stop=(k2 == KI - 1),
                    )
                yo = sb1.tile([128, 512], dt.bfloat16, tag=f"yo{mo % 2}")
                if gate_key is not None:
                    nc.vector.tensor_mul(yo[:], ps2[:], gate_t[gate_key][:])
                else:
                    nc.scalar.activation(yo[:], ps2[:], AF.Copy)
                nc.sync.dma_start(
                    out_dram[mo * 128 : (mo + 1) * 128, out_col : out_col + 512],
                    yo[:],
                )

            return f

        # ---------------- emission schedule ----------------
        # phase 1: shared m1 (32 half-units). Early input DMAs first; the
        # router spread over ui 0..13; AllGather at ui 14; bisection woven
        # from ui 25 (PE arrives there ~115us, after the AG completes) at
        # 3/half-unit, spilling into phase 2; gates/compact/dispatch in the
        # phase-2 weave so their deps are long ready when the PE arrives.
        xsT_dma(0)
        w1s_dma(0)
        xsT_dma(1)

        # shared W2 (sync, phase-1 woven: one 256KB load every other half);
        # expert W1 via the scalar queue mid-phase-1
        wrs2_loads = [
            mk_wload(wrs2, k2, t["ws2_b"], DIM, "wr2s_", nc.sync)
            for k2 in range(KI)
        ]
        wr1_loads = [
            mk_wload(wr1, k, t["w1_b"], I2, "wr1e_", nc.scalar) for k in range(KD)
        ]

        hooks = {
            0: [mk_router_dma(0, [0, 1, 2, 3])],
            2: [mk_router_dma(0, [4, 5, 6, 7])],
            3: [mk_router_mm(0, 0, [0, 1, 2, 3])],
            5: [mk_router_mm(0, 1, [4, 5, 6, 7]), mk_router_fin(0)],
            6: [mk_router_dma(1, [0, 1, 2, 3])],
            8: [mk_router_dma(1, [4, 5, 6, 7])],
            9: [mk_router_mm(1, 0, [0, 1, 2, 3])],
            12: [mk_router_mm(1, 1, [4, 5, 6, 7]), mk_router_fin(1)],
            14: [rt_collective],
            16: [rt_init],
            24: [rt_sig],
        }
        for j, w in enumerate(wr1_loads):
            hooks.setdefault(16 + j, []).append(w)

        bis_units = [mk_bisect(it) for it in range(BISECT_ITERS)]
        bi = iter(bis_units)
        for ui in range(25, 32):
            hooks.setdefault(ui, []).extend(
                u for u in (next(bi, None), next(bi, None), next(bi, None)) if u
            )

        wl = iter(wrs2_loads)
        for ui in range(32):
            i, n = ui // 2, ui % 2
            sh_m1_half(i, n)()
            if n == 0:
                nxt = next(wl, None)
                if nxt is not None:
                    nxt()
            for h in hooks.get(ui, []):
                h()

        # phase 2: shared m2 batch 0 (8 units) with the routing tail woven in
        p2_hooks = {
            0: [u for u in (next(bi, None), next(bi, None), next(bi, None)) if u],
            1: [u for u in (next(bi, None), next(bi, None), next(bi, None)) if u],
            2: [rt_gates],
            3: [rt_compact],
            4: [mk_gather(0)],
            5: [mk_gather(1)],
        }
        for mo in range(KD):
            mk_m2(wrs2, h_sh[0], t["out_shared"], 0, None, mo)()
            for h in p2_hooks.get(mo, []):
                h()
        mk_grep(0)()

        # phase 3: expert FFN, 4 chunks of 512 slots. Expert W2 (aliasing the
        # wrs2 tiles) loads via the scalar queue once shared m2 batch 1 has
        # fully read them.
        wr2_loads = [
            mk_wload(wr2, k2, t["w2_b"], DIM, "wr2s_", nc.scalar)
            for k2 in range(KI)
        ]
        for n in range(SLOTS // 512):
            hnew = sb1.tile([128, KI * 512], dt.bfloat16, tag=f"h_sb{n % 2}")
            for i in range(16):
                mk_m1(n, i, hnew)()
                if n == 0 and i == 2:
                    mk_grep(1)()
            if n == 0:
                # shared m2 batch 1 must fully read the wrs2 tiles before
                # the expert W2 loads overwrite them
                for mo in range(KD):
                    mk_m2(wrs2, h_sh[1], t["out_shared"], 512, None, mo)()
                for w in wr2_loads:
                    w()
            for mo in range(KD):
                mk_m2(wr2, hnew, t["out_routed"], n * 512, f"g{n}", mo)()
            if n + 2 < SLOTS // 512:
                mk_gather(n + 2)()
                mk_grep(n + 2)()


# ======================= host side =======================

_CACHED_NC = None


def _get_nc():
    global _CACHED_NC
    if _CACHED_NC is None:
        _CACHED_NC = build_nc()
    return _CACHED_NC


def make_in_maps(inputs):
    hs_flat = np.ascontiguousarray(
        np.asarray(inputs["hidden_states"], dtype=np.float32).reshape(T, DIM)
    )
    hsu_flat = np.ascontiguousarray(
        np.asarray(inputs["hidden_states_unmodulated"], dtype=np.float32).reshape(
            T, DIM
        )
    )
    ts = np.asarray(inputs["timestep"], dtype=np.float32)
    Wg = np.asarray(inputs["Wg"], dtype=np.float32)
    W1 = np.asarray(inputs["W1"], dtype=np.float32)
    W2 = np.asarray(inputs["W2"], dtype=np.float32)
    Ws1 = np.asarray(inputs["Ws1"], dtype=np.float32)
    Ws2 = np.asarray(inputs["Ws2"], dtype=np.float32)

    lt16 = np.triu(np.ones((16, 16), np.float32), 1)  # lhsT[k,m]=1 iff k<m
    b2 = np.zeros((16, BS), np.float32)
    b2[:8, 0] = 1.0
    b2[8:, 1] = 1.0
    # partition layout: p = r*16 + e*2 + c  (r = source core, e = expert,
    # c = 512-token half of the core's shard)
    p = np.arange(128)
    pb = p // 64  # batch  (r//4)
    pe = (p % 16) // 2  # expert
    rsmat = ((pb[:, None] == pb[None, :]) & (pe[:, None] == pe[None, :])).astype(
        np.float32
    )
    # dsel[p, j]: p belongs to token-group j = r*2 + c (sum over experts)
    j = np.arange(16)
    dsel = ((p[:, None] // 16 == j[None, :] // 2) & (p[:, None] % 2 == j[None, :] % 2)
            ).astype(np.float32)
    jj = np.arange(16)[:, None]
    tt = np.arange(512)[None, :]
    iota_tid = (jj * 512 + tt).astype(np.uint16)
    # router: transposed Wg (hsu half) packed k-major, host-computed ts bias
    wgt = np.ascontiguousarray(Wg[:, DIM:].T)  # [DIM, E]
    wgtp = np.ascontiguousarray(
        wgt.reshape(KD, 128, E).transpose(1, 0, 2).reshape(128, KD * E)
    )
    bias_all = ts @ Wg[:, :DIM].T  # [BS, E]

    import ml_dtypes

    bf16 = ml_dtypes.bfloat16
    hs_b = hs_flat.astype(bf16)
    W1_b = W1.astype(bf16)
    W2_b = W2.astype(bf16)
    # shared W1 packed (a_i | g_i) column pairs
    a = Ws1[:, :INNER].reshape(DIM, 16, 128)
    g = Ws1[:, INNER:].reshape(DIM, 16, 128)
    Ws1p = np.ascontiguousarray(
        np.concatenate([a[:, :, None, :], g[:, :, None, :]], axis=2).reshape(DIM, I2)
    ).astype(bf16)
    Ws2_b = np.ascontiguousarray(Ws2.astype(bf16))
    in_maps = []
    for c in range(NCORES):
        # extract my expert's 16 rows in (b-major, chunk) order:
        # j = r*2 + cc  ->  partition (j//2)*16 + c*2 + (j%2)
        esel = np.zeros((128, 16), np.float32)
        for jx in range(16):
            esel[(jx // 2) * 16 + c * 2 + (jx % 2), jx] = 1.0
        constf = np.zeros((128, 306), np.float32)
        constf[:, C_ESEL : C_ESEL + 16] = esel
        constf[:, C_DSEL : C_DSEL + 16] = dsel
        constf[:16, C_LT16 : C_LT16 + 16] = lt16
        constf[:16, C_B2 : C_B2 + 2] = b2
        constf[:, C_RS : C_RS + 128] = rsmat
        constf[0, C_ONES : C_ONES + 128] = 1.0
        consth = np.zeros((128, 304), np.float16)
        consth[:, 0:128] = rsmat
        consth[:, 128:144] = esel
        consth[:, 144:160] = dsel
        consth[:16, 160:176] = lt16
        consth[0, 176:304] = 1.0
        in_maps.append(
            {
                "hs_b": hs_b,
                "hs_shT_b": np.ascontiguousarray(hs_b[c * TSH : (c + 1) * TSH].T),
                "hsuT_sh": np.ascontiguousarray(
                    hsu_flat[c * TSH : (c + 1) * TSH].T
                ),
                "wgtp": wgtp,
                "bias_mine": np.ascontiguousarray(
                    bias_all[c // 4].reshape(E, 1)
                ),
                "w1_b": np.ascontiguousarray(W1_b[c]),
                "w1p_b": Ws1p,
                "w2_b": np.ascontiguousarray(W2_b[c]),
                "ws2_b": Ws2_b,
                "constf": constf,
                "consth": consth,
                "iota_tid": iota_tid,
            }
        )
    return in_maps


def combine(results):
    out = np.empty((T, DIM), np.float32)
    for c in range(NCORES):
        out[c * TSH : (c + 1) * TSH] = results[c]["out_shared"].T.astype(np.float32)
    for c in range(NCORES):
        idx = results[c]["out_idx"].reshape(SLOTS)
        out[idx] += results[c]["out_routed"].T.astype(np.float32)
    return out.reshape(BS, SLEN, DIM)


def kernel(**inputs):
    nc = _get_nc()
    in_maps = make_in_maps(inputs)
    res = run_bass_kernel_spmd(nc, in_maps, list(range(NCORES))).results
    return combine(res)


if __name__ == "__main__":
    nc = build_nc()
    print("build ok:", len(nc.inst_map), "instructions")
